# revision 8
# baseline (speedup 1.0000x reference)
"""Trainium2 Bass kernel for BatchAll triplet loss.

Reference computation (B=512, D=1024):
    pw = img @ sent.T                                  [B, B]
    t[a,p,n] = pw[a,p] - pw[a,n] + margin
    valid[a,p,n] = (lab[a]==lab[p]) & (lab[a]!=lab[n])
    loss = sum(relu(valid*t)) / (count(valid*t > EPS) + EPS)

Strategy: the batch is class-sorted on the host (a pure permutation of the
(image, sentence, label) triples; the loss is permutation invariant), then
anchors are sharded across 8 cores (64 each, C = core*64). After sorting,
all positives of anchor a live in a contiguous class run inside the core's
128-wide sentence window [C-32, C+96) (holds when max class size <= 33;
dense fallback otherwise). Each core enumerates its actual valid (a,p)
pairs (sum of class sizes over its anchors, ~320 for uniform labels) and
packs them onto partitions: tiles of 128 pairs, free axis = all 512 n.

Per core, with the sentence axis pre-rotated so the window is cols [0,128):
    pw[a,n]   (PE, fp8e5m2 DoubleRow, 4 matmuls)         [64, 512]
    zext[a,n] = pw[a,n] - margin  (UNMASKED, fp16; row 64 = +30000 pad)
    per pair-tile t:
        Z[k,n]  = zext[a_k, n]        (PE one-hot matmul)    [128, 512]
        w[k]    = (Z[k,0:W]+margin) . oneJ  (DVE rowsum)  = pw[a_k,p_k]
        relu(-Z + w) accum -> Sacc    (ACT; includes same-label n)
        count Z < w accum  -> Cacc    (Pool; includes same-label n)
        corrections over the 128-wide window only (same-label n live
        there): relu(w-Z)*eq accum -> corrS, (Z<w)*eq accum -> corrC
Host reduces: loss = (S - corrS) / (C - corrC + EPS).

All DMA rides the single sync HWDGE queue (one queue-drain at exit);
packT is pre-packed partition-major so each partition is one contiguous
descriptor. Raw [128, T]-ish accumulators are DMA'd out; host reduces.
"""

import numpy as np
from contextlib import ExitStack

B = 512
D = 1024
NCORES = 8
A = B // NCORES   # 64 anchors per core
KT = D // 128     # 8 contraction tiles
NT = B // 128     # 4 n-tiles per anchor (dense variant)
W = 128           # per-core sentence window width
MARGIN = 0.2
EPS = 1e-16
BIG = 1e30
BIGW = 30000.0
MAXC_WIN = 33     # pair variant valid iff max class size <= this
FP8 = True        # embeddings in fp8e5m2: halves the packT DMA

_CACHE = {}


def _build_pairs(T):
    """Pair-packed kernel: T tiles of 128 (anchor, positive) pairs."""
    import concourse.mybir as mybir
    import concourse.tile as tile
    from concourse import bacc

    f32 = mybir.dt.float32
    f16 = mybir.dt.float16
    Alu = mybir.AluOpType
    Act = mybir.ActivationFunctionType

    nc = bacc.Bacc("TRN2", target_bir_lowering=False, debug=False,
                   num_devices=NCORES)

    K = T * 128
    f8 = mybir.dt.float8e5
    # packT partition-major: each partition's 8 k-tile chunks contiguous
    packT_d = nc.dram_tensor("packT", [128, KT, A + B], f8 if FP8 else f16,
                             kind="ExternalInput")
    # aux: cols [0,K) oneJ | [K,2K) eqwin (both 128 rows) | [2K,3K) selA
    # (rows 0..64) -- transferred as two tight slices
    aux_d = nc.dram_tensor("aux", [128, 3 * K], f16, kind="ExternalInput")
    outS_d = nc.dram_tensor("outS", [128, T], f32, kind="ExternalOutput")
    outX_d = nc.dram_tensor("outX", [128, 3 * T], f32, kind="ExternalOutput")

    with tile.TileContext(nc) as tc:
        with ExitStack() as ctx:
            spool = ctx.enter_context(tc.tile_pool(name="spool", bufs=1))
            ppool = ctx.enter_context(
                tc.tile_pool(name="ppool", bufs=1, space="PSUM"))
            zpool = ctx.enter_context(
                tc.tile_pool(name="zpool", bufs=3, space="PSUM"))

            # ---- all input DMA on the single sync HWDGE queue; selA slice
            # first (z0 needs it right after pw), then oneJ+eqwin ----
            packT = spool.tile([128, KT, A + B], f8 if FP8 else f16)
            nc.sync.dma_start(out=packT[:, 0:4, :], in_=packT_d.ap()[:, 0:4, :])
            nc.sync.dma_start(out=packT[:, 4:8, :], in_=packT_d.ap()[:, 4:8, :])
            aux = spool.tile([128, 3 * K], f16)
            nc.sync.dma_start(out=aux[0:A + 1, 2 * K:3 * K],
                              in_=aux_d.ap()[0:A + 1, 2 * K:3 * K])
            nc.sync.dma_start(out=aux[:, 0:2 * K], in_=aux_d.ap()[:, 0:2 * K])
            oneJ = aux[:, 0:K]
            eqwin = aux[:, K:2 * K]
            selA = aux[0:A + 1, 2 * K:3 * K]
            imgT = packT[:, :, 0:A]
            sentT = packT[:, :, A:A + B]

            # ---- accumulators: one tile per writer engine (Tile tracks
            # hazards at tile granularity; cross-engine writers to one tile
            # would serialize) ----
            Sacc = spool.tile([128, T], f32)      # Scalar
            corr = spool.tile([128, 3 * T], f32)  # Vector: corrS|corrC|count
            wcol = spool.tile([128, T], f32)      # Vector

            # ---- pairwise rows (sentT pre-rotated: window = cols 0..W).
            # fp8 DoubleRow folds two k-tiles into each matmul ----
            pw_ps = ppool.tile([A, B], f32)
            if FP8:
                for u in range(KT // 2):
                    nc.tensor.matmul(
                        pw_ps, lhsT=imgT[:, 2 * u:2 * u + 2, :],
                        rhs=sentT[:, 2 * u:2 * u + 2, :],
                        start=(u == 0), stop=(u == KT // 2 - 1),
                        perf_mode=mybir.MatmulPerfMode.DoubleRow)
            else:
                for kt in range(KT):
                    nc.tensor.matmul(pw_ps, lhsT=imgT[:, kt, :],
                                     rhs=sentT[:, kt, :],
                                     start=(kt == 0), stop=(kt == KT - 1))

            # ---- z rows UNMASKED: zext[a,n] = pw[a,n] - margin (fp16 keeps
            # full pw precision; the label mask is applied via the window
            # corrections below). Row 64 = +BIGW for padded pairs ----
            zext = spool.tile([A + 1, B], f16)
            nc.vector.memset(zext[A:A + 1, :], BIGW)
            nc.vector.tensor_scalar(zext[0:A, :], pw_ps, -MARGIN, None,
                                    Alu.add)

            # ---- main loop ----
            for t in range(T):
                sl = selA[:, t * 128:(t + 1) * 128]
                z_ps = zpool.tile([128, B], f32)
                nc.tensor.matmul(z_ps, lhsT=sl, rhs=zext)
                # w[k] = pw[a_k, p_k] via the window gather: (z+margin).oneJ
                junk = spool.tile([128, W], f16)
                nc.vector.scalar_tensor_tensor(
                    junk, z_ps[:, 0:W], MARGIN,
                    oneJ[:, t * W:(t + 1) * W],
                    Alu.add, Alu.mult,
                    accum_out=wcol[:, t:t + 1])
                # sum(relu(w - z)) over all 512 n (masked terms included,
                # subtracted by corrS below)
                r = spool.tile([128, B], f16)
                nc.scalar.activation(
                    out=r, in_=z_ps, func=Act.Relu,
                    bias=wcol[:, t:t + 1], scale=-1.0,
                    accum_out=Sacc[:, t:t + 1])
                # count r > 0 over all 512 n (masked terms included,
                # subtracted by corrC below)
                m = spool.tile([128, B], f16)
                nc.vector.tensor_scalar(
                    m, r, 0.0, None, Alu.is_gt, Alu.add,
                    accum_out=corr[:, 2 * T + t:2 * T + t + 1])
                # corrections from the same relu output (exact cancel):
                # same-label n all live in the window
                cs = spool.tile([128, W], f16)
                nc.vector.scalar_tensor_tensor(
                    cs, r[:, 0:W], 1.0, eqwin[:, t * W:(t + 1) * W],
                    Alu.mult, Alu.mult,
                    accum_out=corr[:, t:t + 1])
                cc = spool.tile([128, W], f16)
                nc.vector.scalar_tensor_tensor(
                    cc, r[:, 0:W], 0.0, eqwin[:, t * W:(t + 1) * W],
                    Alu.is_gt, Alu.mult,
                    accum_out=corr[:, T + t:T + t + 1])

            # ---- ship raw accumulators; host does the final reduction ----
            nc.sync.dma_start(out=outX_d.ap(), in_=corr)
            nc.sync.dma_start(out=outS_d.ap(), in_=Sacc)

    nc.compile()
    return nc


def _build_dense():
    """Dense fallback (no class-size assumption)."""
    import concourse.mybir as mybir
    import concourse.tile as tile
    from concourse import bacc
    from concourse.masks import make_identity

    f32 = mybir.dt.float32
    bf16 = mybir.dt.bfloat16
    Alu = mybir.AluOpType
    Act = mybir.ActivationFunctionType
    Ax = mybir.AxisListType

    nc = bacc.Bacc("TRN2", target_bir_lowering=False, debug=False,
                   num_devices=NCORES)

    imgT_d = nc.dram_tensor("imgT", [D, A], f32, kind="ExternalInput")
    sentT_d = nc.dram_tensor("sentT", [D, B], f32, kind="ExternalInput")
    labf_d = nc.dram_tensor("labf", [B], bf16, kind="ExternalInput")
    labc_d = nc.dram_tensor("labc", [A], f32, kind="ExternalInput")
    out_d = nc.dram_tensor("out", [2], f32, kind="ExternalOutput")

    with tile.TileContext(nc) as tc:
        with ExitStack() as ctx:
            singles = ctx.enter_context(tc.tile_pool(name="singles", bufs=1))
            rpool = ctx.enter_context(tc.tile_pool(name="rpool", bufs=6))
            mpool = ctx.enter_context(tc.tile_pool(name="mpool", bufs=6))
            spsum = ctx.enter_context(
                tc.tile_pool(name="spsum", bufs=1, space="PSUM"))
            wpsum = ctx.enter_context(
                tc.tile_pool(name="wpsum", bufs=3, space="PSUM"))
            gpsum = ctx.enter_context(
                tc.tile_pool(name="gpsum", bufs=2, space="PSUM"))

            ones_r = singles.tile([1, 128], f32)
            nc.vector.memset(ones_r, 1.0)
            ones_c = singles.tile([128, 1], f32)
            nc.vector.memset(ones_c, 1.0)
            ident = singles.tile([64, 64], f32)
            make_identity(nc, ident)

            imgT = singles.tile([128, KT, A], f32)
            nc.sync.dma_start(
                out=imgT, in_=imgT_d.ap().rearrange("(t p) m -> p t m", p=128))
            sentT = singles.tile([128, KT, B], f32)
            nc.sync.dma_start(
                out=sentT, in_=sentT_d.ap().rearrange("(t p) m -> p t m", p=128))
            lab_row = singles.tile([1, B], f32)
            nc.sync.dma_start(
                out=lab_row, in_=labf_d.ap().rearrange("(o b) -> o b", o=1))
            labc_col = singles.tile([A, 1], f32)
            nc.sync.dma_start(
                out=labc_col, in_=labc_d.ap().rearrange("(a o) -> a o", o=1))

            pw_ps = spsum.tile([A, B], f32)
            for kt in range(KT):
                nc.tensor.matmul(pw_ps, lhsT=imgT[:, kt, :], rhs=sentT[:, kt, :],
                                 start=(kt == 0), stop=(kt == KT - 1))

            labB_ps = spsum.tile([A, B], f32)
            nc.tensor.matmul(labB_ps, lhsT=ones_r[:, :A], rhs=lab_row)
            eqP = singles.tile([A, B], f32)
            nc.vector.tensor_scalar(eqP, labB_ps, labc_col, None, Alu.is_equal)
            penP = singles.tile([A, B], f32)
            nc.vector.tensor_scalar(penP, eqP, 1.0, BIG, Alu.subtract, Alu.mult)
            penN = singles.tile([A, B], f32)
            nc.vector.tensor_scalar(penN, eqP, -BIG, None, Alu.mult)

            w = singles.tile([A, B], f32)
            nc.vector.tensor_scalar(w, pw_ps, MARGIN, None, Alu.add)
            nc.vector.tensor_mul(w, w, eqP)
            nc.vector.tensor_add(w, w, penP)
            negneq = singles.tile([A, B], f32)
            nc.vector.tensor_scalar(negneq, eqP, 1.0, -1.0, Alu.subtract,
                                    Alu.mult)
            z = singles.tile([A, B], f32)
            nc.vector.tensor_scalar(z, pw_ps, -1.0, None, Alu.mult)
            nc.vector.tensor_mul(z, z, negneq)
            nc.vector.tensor_add(z, z, penN)

            zTs = singles.tile([128, NT, A], f32)
            for j in range(NT):
                zt_ps = spsum.tile([128, A], f32)
                nc.tensor.transpose(zt_ps, z[:, j * 128:(j + 1) * 128], ident)
                nc.scalar.copy(zTs[:, j, :], zt_ps)

            Sacc = singles.tile([128, A * NT], f32)
            Cacc = singles.tile([128, A * NT], f32)

            for a in range(A):
                wb_ps = wpsum.tile([128, B], f32)
                nc.tensor.matmul(
                    wb_ps, lhsT=ident[:, a:a + 1].broadcast_to([A, 128]), rhs=w)
                for j in range(NT):
                    col = a * NT + j
                    r = rpool.tile([128, B], bf16)
                    nc.scalar.activation(
                        out=r, in_=wb_ps, func=Act.Relu,
                        bias=zTs[:, j, a:a + 1], scale=1.0,
                        accum_out=Sacc[:, col:col + 1])
                    m = mpool.tile([128, B], bf16)
                    nc.vector.tensor_scalar(
                        m, r, EPS, None, Alu.is_gt, Alu.add,
                        accum_out=Cacc[:, col:col + 1])

            SC = singles.tile([128, 2], f32)
            nc.vector.tensor_reduce(SC[:, 0:1], Sacc, Ax.X, Alu.add)
            nc.vector.tensor_reduce(SC[:, 1:2], Cacc, Ax.X, Alu.add)
            fin_ps = spsum.tile([2, 1], f32)
            nc.tensor.matmul(fin_ps, lhsT=SC, rhs=ones_c)
            fin_sb = singles.tile([2, 1], f32)
            nc.scalar.copy(fin_sb, fin_ps)
            nc.sync.dma_start(
                out=out_d.ap().rearrange("(p o) -> p o", o=1), in_=fin_sb)

    nc.compile()
    return nc


def _get_nc(variant, T=0):
    key = f"nc_{variant}_{T}"
    if key not in _CACHE:
        _CACHE[key] = (_build_pairs(T) if variant == "pairs"
                       else _build_dense())
    return _CACHE[key]


def _prep(labels, image_embeddings, sentence_embeddings):
    """Class-sort the batch; build per-core input maps."""
    labels = np.ascontiguousarray(labels).astype(np.int64)
    img = np.ascontiguousarray(image_embeddings, dtype=np.float32)
    sent = np.ascontiguousarray(sentence_embeddings, dtype=np.float32)
    counts = np.bincount(labels, minlength=1)
    maxc = counts.max()

    perm = np.argsort(labels, kind="stable")
    labs = labels[perm]

    if maxc > MAXC_WIN:
        imgT = np.ascontiguousarray(img[perm].T)    # [D, B]
        sentT = np.ascontiguousarray(sent[perm].T)  # [D, B]
        labsf = labs.astype(np.float32)
        maps = []
        for i in range(NCORES):
            c0 = i * A
            maps.append({
                "imgT": np.ascontiguousarray(imgT[:, c0:c0 + A]),
                "sentT": sentT,
                "labf": labsf,
                "labc": np.ascontiguousarray(labsf[c0:c0 + A]),
            })
        return "dense", 0, maps

    if FP8:
        import ml_dtypes
        edt = ml_dtypes.float8_e5m2
    else:
        edt = np.float16
    imgT = np.ascontiguousarray(img[perm].T).astype(edt)
    sentT = np.ascontiguousarray(sent[perm].T).astype(edt)

    # class run start/size per sorted position
    starts = np.concatenate([[0], np.cumsum(counts)])
    s_a = starts[labs]            # run start of each anchor
    n_a = counts[labs]            # run length of each anchor
    maxK = max(int(n_a[c0:c0 + A].sum()) for c0 in range(0, B, A))
    T = (maxK + 127) // 128
    K = T * 128

    maps = []
    for i in range(NCORES):
        c0 = i * A
        rot = (np.arange(B) + c0 - 32) % B
        # partition-major packT: [p, t, m] = flat[(t*128+p), m]
        flat = np.concatenate([imgT[:, c0:c0 + A], sentT[:, rot]], axis=1)
        packT = np.ascontiguousarray(
            flat.reshape(KT, 128, A + B).transpose(1, 0, 2))
        # aux layout: [128, K] oneJ | [128, K] eqwin | [65, K] selA
        aux = np.zeros((128, 3 * K), np.float16)
        eq = labs[rot[:W]][None, :] == labs[c0:c0 + A][:, None]  # [A, W]
        k = 0
        for a in range(A):
            ga = c0 + a
            for p in range(int(s_a[ga]), int(s_a[ga] + n_a[ga])):
                j = p - (c0 - 32)
                aux[k % 128, (k // 128) * W + j] = 1.0        # oneJ
                aux[k % 128, K + (k // 128) * W:K + (k // 128) * W + W] = \
                    eq[a].astype(np.float16)                  # eqwin row
                aux[a, 2 * K + k] = 1.0                       # selA
                k += 1
        aux[A, 2 * K + k:3 * K] = 1.0   # pads select zext row 64 (+BIGW)
        maps.append({"packT": packT, "aux": aux})
    return "pairs", T, maps


def run_all(labels, image_embeddings, sentence_embeddings, trace=False):
    from concourse.bass_utils import run_bass_kernel_spmd
    variant, T, maps = _prep(labels, image_embeddings, sentence_embeddings)
    nc = _get_nc(variant, T)
    res = run_bass_kernel_spmd(nc, maps, list(range(NCORES)), trace=trace)
    if variant == "pairs":
        s = c = 0.0
        for i in range(NCORES):
            x = res.results[i]["outX"]
            s += float(res.results[i]["outS"].sum()) - float(x[:, 0:T].sum())
            c += float(x[:, 2 * T:].sum()) - float(x[:, T:2 * T].sum())
    else:
        parts = np.stack([res.results[i]["out"] for i in range(NCORES)])
        s = float(parts[:, 0].sum())
        c = float(parts[:, 1].sum())
    loss = np.float32(s / (c + EPS))
    return np.asarray(loss, dtype=np.float32), res


def kernel(labels, image_embeddings, sentence_embeddings):
    out, _ = run_all(labels, image_embeddings, sentence_embeddings)
    return out


# revision 9
# speedup vs baseline: 1.1094x; 1.1094x over previous
"""Trainium2 Bass kernel for BatchAll triplet loss.

Reference computation (B=512, D=1024):
    pw = img @ sent.T                                  [B, B]
    t[a,p,n] = pw[a,p] - pw[a,n] + margin
    valid[a,p,n] = (lab[a]==lab[p]) & (lab[a]!=lab[n])
    loss = sum(relu(valid*t)) / (count(valid*t > EPS) + EPS)

Strategy: the batch is class-sorted on the host (a pure permutation of the
(image, sentence, label) triples; the loss is permutation invariant), then
anchors are sharded across 8 cores (64 each, C = core*64). After sorting,
all positives of anchor a live in a contiguous class run inside the core's
128-wide sentence window [C-32, C+96) (holds when max class size <= 33;
dense fallback otherwise). Each core enumerates its actual valid (a,p)
pairs (sum of class sizes over its anchors, ~320 for uniform labels) and
packs them onto partitions: tiles of 128 pairs, free axis = all 512 n.

Per core, with the sentence axis pre-rotated so the window is cols [0,128):
    pw[a,n]   (PE, fp8e5m2 DoubleRow, 4 matmuls)         [64, 512]
    zext[a,n] = pw[a,n] - margin  (UNMASKED, fp16; row 64 = +30000 pad)
    per pair-tile t:
        Z[k,n]  = zext[a_k, n]        (PE one-hot matmul)    [128, 512]
        w[k]    = (Z[k,0:W]+margin) . oneJ  (DVE rowsum)  = pw[a_k,p_k]
        relu(-Z + w) accum -> Sacc    (ACT; includes same-label n)
        count Z < w accum  -> Cacc    (Pool; includes same-label n)
        corrections over the 128-wide window only (same-label n live
        there): relu(w-Z)*eq accum -> corrS, (Z<w)*eq accum -> corrC
Host reduces: loss = (S - corrS) / (C - corrC + EPS).

All DMA rides the single sync HWDGE queue (one queue-drain at exit);
packT is pre-packed partition-major so each partition is one contiguous
descriptor. Raw [128, T]-ish accumulators are DMA'd out; host reduces.
"""

import numpy as np
from contextlib import ExitStack

B = 512
D = 1024
NCORES = 8
A = B // NCORES   # 64 anchors per core
KT = D // 128     # 8 contraction tiles
NT = B // 128     # 4 n-tiles per anchor (dense variant)
W = 128           # per-core sentence window width
MARGIN = 0.2
EPS = 1e-16
BIG = 1e30
BIGW = 30000.0
MAXC_WIN = 33     # pair variant valid iff max class size <= this
FP8 = True        # embeddings in fp8e5m2: halves the packT DMA

_CACHE = {}


def _build_pairs(T):
    """Pair-packed kernel: T tiles of 128 (anchor, positive) pairs."""
    import concourse.mybir as mybir
    import concourse.tile as tile
    from concourse import bacc

    f32 = mybir.dt.float32
    f16 = mybir.dt.float16
    Alu = mybir.AluOpType
    Act = mybir.ActivationFunctionType

    nc = bacc.Bacc("TRN2", target_bir_lowering=False, debug=False,
                   num_devices=NCORES)

    K = T * 128
    f8 = mybir.dt.float8e5
    # packT partition-major: each partition's 8 k-tile chunks contiguous
    packT_d = nc.dram_tensor("packT", [128, KT, A + B], f8 if FP8 else f16,
                             kind="ExternalInput")
    # aux regions along dim1: [0,T) oneJ | [T,2T) eqwin | [2T,3T) selA
    # (selA uses rows 0..64 only)
    aux_d = nc.dram_tensor("aux", [128, 3 * T, W], f16, kind="ExternalInput")
    # out cols: [0,T) Sacc | T corrS | T+1 corrC | T+2 count
    out_d = nc.dram_tensor("out", [128, T + 3], f32, kind="ExternalOutput")

    with tile.TileContext(nc) as tc:
        with ExitStack() as ctx:
            spool = ctx.enter_context(tc.tile_pool(name="spool", bufs=1))
            ppool = ctx.enter_context(
                tc.tile_pool(name="ppool", bufs=1, space="PSUM"))
            zpool = ctx.enter_context(
                tc.tile_pool(name="zpool", bufs=3, space="PSUM"))

            # ---- packT on the sync HWDGE queue; aux on the gpsimd SWDGE
            # queue so the issue instructions run in parallel ----
            packT = spool.tile([128, KT, A + B], f8 if FP8 else f16)
            nc.sync.dma_start(out=packT[:, 0:4, :], in_=packT_d.ap()[:, 0:4, :])
            nc.sync.dma_start(out=packT[:, 4:8, :], in_=packT_d.ap()[:, 4:8, :])
            aux = spool.tile([128, 3 * T, W], f16)
            nc.gpsimd.dma_start(out=aux, in_=aux_d.ap())
            imgT = packT[:, :, 0:A]
            sentT = packT[:, :, A:A + B]

            # single accumulator tile: Scalar writes cols [0,T) during the
            # loop, Vector writes cols [T,T+3) strictly after -- no
            # interleaved cross-engine writes, so no false serialization
            SC = spool.tile([128, T + 3], f32)
            wcol = spool.tile([128, T], f32)

            # zext pad row on gpsimd, then a dummy relu on Scalar pulls the
            # ACT table load off the critical path (it otherwise sits
            # behind the first real relu's semaphore wait)
            zext = spool.tile([A + 1, B], f16)
            nc.gpsimd.memset(zext[A:A + 1, :], BIGW)
            dum = spool.tile([1, 1], f16)
            nc.scalar.activation(out=dum, in_=zext[A:A + 1, 0:1],
                                 func=Act.Relu, bias=0.0, scale=1.0)

            # ---- pairwise rows (sentT pre-rotated: window = cols 0..W).
            # fp8 DoubleRow folds two k-tiles into each matmul ----
            pw_ps = ppool.tile([A, B], f32)
            if FP8:
                for u in range(KT // 2):
                    nc.tensor.matmul(
                        pw_ps, lhsT=imgT[:, 2 * u:2 * u + 2, :],
                        rhs=sentT[:, 2 * u:2 * u + 2, :],
                        start=(u == 0), stop=(u == KT // 2 - 1),
                        perf_mode=mybir.MatmulPerfMode.DoubleRow)
            else:
                for kt in range(KT):
                    nc.tensor.matmul(pw_ps, lhsT=imgT[:, kt, :],
                                     rhs=sentT[:, kt, :],
                                     start=(kt == 0), stop=(kt == KT - 1))

            # ---- z rows UNMASKED: zext[a,n] = pw[a,n] - margin (fp16 keeps
            # full pw precision; the label mask is applied via the window
            # corrections below). On Scalar: Vector stays free for the loop
            nc.scalar.activation(out=zext[0:A, :], in_=pw_ps, func=Act.Copy,
                                 bias=-MARGIN, scale=1.0)

            # ---- main loop: z matmul -> w gather (Vector) -> relu (Scalar)
            r_all = spool.tile([128, T, B], f16)
            for t in range(T):
                z_ps = zpool.tile([128, B], f32)
                nc.tensor.matmul(z_ps, lhsT=aux[0:A + 1, 2 * T + t, :],
                                 rhs=zext)
                # w[k] = pw[a_k, p_k] via the window gather: (z+margin).oneJ
                junk = spool.tile([128, W], f16)
                nc.vector.scalar_tensor_tensor(
                    junk, z_ps[:, 0:W], MARGIN, aux[:, t, :],
                    Alu.add, Alu.mult,
                    accum_out=wcol[:, t:t + 1])
                # sum(relu(w - z)) over all 512 n (masked terms included,
                # corrected below)
                nc.scalar.activation(
                    out=r_all[:, t, :], in_=z_ps, func=Act.Relu,
                    bias=wcol[:, t:t + 1], scale=-1.0,
                    accum_out=SC[:, t:t + 1])

            # ---- single-shot reductions over all T tiles of relu output:
            # corrS/corrC subtract the same-label (window) terms, count is
            # the raw positive count ----
            eqw = aux[:, T:2 * T, :]
            rwin = r_all[:, :, 0:W]
            csj = spool.tile([128, T, W], f16)
            nc.vector.scalar_tensor_tensor(
                csj, rwin, 1.0, eqw, Alu.mult, Alu.mult,
                accum_out=SC[:, T:T + 1])
            ccj = spool.tile([128, T, W], f16)
            nc.vector.scalar_tensor_tensor(
                ccj, rwin, 0.0, eqw, Alu.is_gt, Alu.mult,
                accum_out=SC[:, T + 1:T + 2])
            mj = spool.tile([128, T, B], f16)
            nc.vector.tensor_scalar(
                mj, r_all, 0.0, None, Alu.is_gt, Alu.add,
                accum_out=SC[:, T + 2:T + 3])

            # ---- ship raw accumulators; host does the final reduction ----
            nc.sync.dma_start(out=out_d.ap(), in_=SC)

    nc.compile()
    return nc


def _build_dense():
    """Dense fallback (no class-size assumption)."""
    import concourse.mybir as mybir
    import concourse.tile as tile
    from concourse import bacc
    from concourse.masks import make_identity

    f32 = mybir.dt.float32
    bf16 = mybir.dt.bfloat16
    Alu = mybir.AluOpType
    Act = mybir.ActivationFunctionType
    Ax = mybir.AxisListType

    nc = bacc.Bacc("TRN2", target_bir_lowering=False, debug=False,
                   num_devices=NCORES)

    imgT_d = nc.dram_tensor("imgT", [D, A], f32, kind="ExternalInput")
    sentT_d = nc.dram_tensor("sentT", [D, B], f32, kind="ExternalInput")
    labf_d = nc.dram_tensor("labf", [B], bf16, kind="ExternalInput")
    labc_d = nc.dram_tensor("labc", [A], f32, kind="ExternalInput")
    out_d = nc.dram_tensor("out", [2], f32, kind="ExternalOutput")

    with tile.TileContext(nc) as tc:
        with ExitStack() as ctx:
            singles = ctx.enter_context(tc.tile_pool(name="singles", bufs=1))
            rpool = ctx.enter_context(tc.tile_pool(name="rpool", bufs=6))
            mpool = ctx.enter_context(tc.tile_pool(name="mpool", bufs=6))
            spsum = ctx.enter_context(
                tc.tile_pool(name="spsum", bufs=1, space="PSUM"))
            wpsum = ctx.enter_context(
                tc.tile_pool(name="wpsum", bufs=3, space="PSUM"))
            gpsum = ctx.enter_context(
                tc.tile_pool(name="gpsum", bufs=2, space="PSUM"))

            ones_r = singles.tile([1, 128], f32)
            nc.vector.memset(ones_r, 1.0)
            ones_c = singles.tile([128, 1], f32)
            nc.vector.memset(ones_c, 1.0)
            ident = singles.tile([64, 64], f32)
            make_identity(nc, ident)

            imgT = singles.tile([128, KT, A], f32)
            nc.sync.dma_start(
                out=imgT, in_=imgT_d.ap().rearrange("(t p) m -> p t m", p=128))
            sentT = singles.tile([128, KT, B], f32)
            nc.sync.dma_start(
                out=sentT, in_=sentT_d.ap().rearrange("(t p) m -> p t m", p=128))
            lab_row = singles.tile([1, B], f32)
            nc.sync.dma_start(
                out=lab_row, in_=labf_d.ap().rearrange("(o b) -> o b", o=1))
            labc_col = singles.tile([A, 1], f32)
            nc.sync.dma_start(
                out=labc_col, in_=labc_d.ap().rearrange("(a o) -> a o", o=1))

            pw_ps = spsum.tile([A, B], f32)
            for kt in range(KT):
                nc.tensor.matmul(pw_ps, lhsT=imgT[:, kt, :], rhs=sentT[:, kt, :],
                                 start=(kt == 0), stop=(kt == KT - 1))

            labB_ps = spsum.tile([A, B], f32)
            nc.tensor.matmul(labB_ps, lhsT=ones_r[:, :A], rhs=lab_row)
            eqP = singles.tile([A, B], f32)
            nc.vector.tensor_scalar(eqP, labB_ps, labc_col, None, Alu.is_equal)
            penP = singles.tile([A, B], f32)
            nc.vector.tensor_scalar(penP, eqP, 1.0, BIG, Alu.subtract, Alu.mult)
            penN = singles.tile([A, B], f32)
            nc.vector.tensor_scalar(penN, eqP, -BIG, None, Alu.mult)

            w = singles.tile([A, B], f32)
            nc.vector.tensor_scalar(w, pw_ps, MARGIN, None, Alu.add)
            nc.vector.tensor_mul(w, w, eqP)
            nc.vector.tensor_add(w, w, penP)
            negneq = singles.tile([A, B], f32)
            nc.vector.tensor_scalar(negneq, eqP, 1.0, -1.0, Alu.subtract,
                                    Alu.mult)
            z = singles.tile([A, B], f32)
            nc.vector.tensor_scalar(z, pw_ps, -1.0, None, Alu.mult)
            nc.vector.tensor_mul(z, z, negneq)
            nc.vector.tensor_add(z, z, penN)

            zTs = singles.tile([128, NT, A], f32)
            for j in range(NT):
                zt_ps = spsum.tile([128, A], f32)
                nc.tensor.transpose(zt_ps, z[:, j * 128:(j + 1) * 128], ident)
                nc.scalar.copy(zTs[:, j, :], zt_ps)

            Sacc = singles.tile([128, A * NT], f32)
            Cacc = singles.tile([128, A * NT], f32)

            for a in range(A):
                wb_ps = wpsum.tile([128, B], f32)
                nc.tensor.matmul(
                    wb_ps, lhsT=ident[:, a:a + 1].broadcast_to([A, 128]), rhs=w)
                for j in range(NT):
                    col = a * NT + j
                    r = rpool.tile([128, B], bf16)
                    nc.scalar.activation(
                        out=r, in_=wb_ps, func=Act.Relu,
                        bias=zTs[:, j, a:a + 1], scale=1.0,
                        accum_out=Sacc[:, col:col + 1])
                    m = mpool.tile([128, B], bf16)
                    nc.vector.tensor_scalar(
                        m, r, EPS, None, Alu.is_gt, Alu.add,
                        accum_out=Cacc[:, col:col + 1])

            SC = singles.tile([128, 2], f32)
            nc.vector.tensor_reduce(SC[:, 0:1], Sacc, Ax.X, Alu.add)
            nc.vector.tensor_reduce(SC[:, 1:2], Cacc, Ax.X, Alu.add)
            fin_ps = spsum.tile([2, 1], f32)
            nc.tensor.matmul(fin_ps, lhsT=SC, rhs=ones_c)
            fin_sb = singles.tile([2, 1], f32)
            nc.scalar.copy(fin_sb, fin_ps)
            nc.sync.dma_start(
                out=out_d.ap().rearrange("(p o) -> p o", o=1), in_=fin_sb)

    nc.compile()
    return nc


def _get_nc(variant, T=0):
    key = f"nc_{variant}_{T}"
    if key not in _CACHE:
        _CACHE[key] = (_build_pairs(T) if variant == "pairs"
                       else _build_dense())
    return _CACHE[key]


def _prep(labels, image_embeddings, sentence_embeddings):
    """Class-sort the batch; build per-core input maps."""
    labels = np.ascontiguousarray(labels).astype(np.int64)
    img = np.ascontiguousarray(image_embeddings, dtype=np.float32)
    sent = np.ascontiguousarray(sentence_embeddings, dtype=np.float32)
    counts = np.bincount(labels, minlength=1)
    maxc = counts.max()

    perm = np.argsort(labels, kind="stable")
    labs = labels[perm]

    if maxc > MAXC_WIN:
        imgT = np.ascontiguousarray(img[perm].T)    # [D, B]
        sentT = np.ascontiguousarray(sent[perm].T)  # [D, B]
        labsf = labs.astype(np.float32)
        maps = []
        for i in range(NCORES):
            c0 = i * A
            maps.append({
                "imgT": np.ascontiguousarray(imgT[:, c0:c0 + A]),
                "sentT": sentT,
                "labf": labsf,
                "labc": np.ascontiguousarray(labsf[c0:c0 + A]),
            })
        return "dense", 0, maps

    if FP8:
        import ml_dtypes
        edt = ml_dtypes.float8_e5m2
    else:
        edt = np.float16
    imgT = np.ascontiguousarray(img[perm].T).astype(edt)
    sentT = np.ascontiguousarray(sent[perm].T).astype(edt)

    # class run start/size per sorted position
    starts = np.concatenate([[0], np.cumsum(counts)])
    s_a = starts[labs]            # run start of each anchor
    n_a = counts[labs]            # run length of each anchor
    maxK = max(int(n_a[c0:c0 + A].sum()) for c0 in range(0, B, A))
    T = (maxK + 127) // 128
    K = T * 128

    maps = []
    for i in range(NCORES):
        c0 = i * A
        rot = (np.arange(B) + c0 - 32) % B
        # partition-major packT: [p, t, m] = flat[(t*128+p), m]
        flat = np.concatenate([imgT[:, c0:c0 + A], sentT[:, rot]], axis=1)
        packT = np.ascontiguousarray(
            flat.reshape(KT, 128, A + B).transpose(1, 0, 2))
        # aux layout: [128, K] oneJ | [128, K] eqwin | [65, K] selA
        aux = np.zeros((128, 3 * K), np.float16)
        eq = labs[rot[:W]][None, :] == labs[c0:c0 + A][:, None]  # [A, W]
        k = 0
        for a in range(A):
            ga = c0 + a
            for p in range(int(s_a[ga]), int(s_a[ga] + n_a[ga])):
                j = p - (c0 - 32)
                aux[k % 128, (k // 128) * W + j] = 1.0        # oneJ
                aux[k % 128, K + (k // 128) * W:K + (k // 128) * W + W] = \
                    eq[a].astype(np.float16)                  # eqwin row
                aux[a, 2 * K + k] = 1.0                       # selA
                k += 1
        aux[A, 2 * K + k:3 * K] = 1.0   # pads select zext row 64 (+BIGW)
        maps.append({"packT": packT, "aux": aux.reshape(128, 3 * T, W)})
    return "pairs", T, maps


def run_all(labels, image_embeddings, sentence_embeddings, trace=False):
    from concourse.bass_utils import run_bass_kernel_spmd
    variant, T, maps = _prep(labels, image_embeddings, sentence_embeddings)
    nc = _get_nc(variant, T)
    res = run_bass_kernel_spmd(nc, maps, list(range(NCORES)), trace=trace)
    if variant == "pairs":
        s = c = 0.0
        for i in range(NCORES):
            x = res.results[i]["out"]
            s += float(x[:, 0:T].sum()) - float(x[:, T].sum())
            c += float(x[:, T + 2].sum()) - float(x[:, T + 1].sum())
    else:
        parts = np.stack([res.results[i]["out"] for i in range(NCORES)])
        s = float(parts[:, 0].sum())
        c = float(parts[:, 1].sum())
    loss = np.float32(s / (c + EPS))
    return np.asarray(loss, dtype=np.float32), res


def kernel(labels, image_embeddings, sentence_embeddings):
    out, _ = run_all(labels, image_embeddings, sentence_embeddings)
    return out


# revision 16
# speedup vs baseline: 1.1938x; 1.0761x over previous
"""Trainium2 Bass kernel for BatchAll triplet loss.

Reference computation (B=512, D=1024):
    pw = img @ sent.T                                  [B, B]
    t[a,p,n] = pw[a,p] - pw[a,n] + margin
    valid[a,p,n] = (lab[a]==lab[p]) & (lab[a]!=lab[n])
    loss = sum(relu(valid*t)) / (count(valid*t > EPS) + EPS)

Strategy: the batch is class-sorted on the host (a pure permutation of the
(image, sentence, label) triples; the loss is permutation invariant), then
anchors are sharded across 8 cores (64 each, C = core*64). After sorting,
all positives of anchor a live in a contiguous class run inside the core's
128-wide sentence window [C-32, C+96) (holds when max class size <= 33;
dense fallback otherwise). Each core enumerates its actual valid (a,p)
pairs (sum of class sizes over its anchors, ~320 for uniform labels) and
packs them onto partitions: tiles of 128 pairs, free axis = all 512 n.

Per core, with the sentence axis pre-rotated so the window is cols [0,128):
    pw[a,n]   (PE, fp8e5m2 DoubleRow, 4 matmuls)         [64, 512]
    zext[a,n] = pw[a,n] - margin  (UNMASKED, fp16; row 64 = +30000 pad)
    per pair-tile t:
        Z[k,n]  = zext[a_k, n]        (PE one-hot matmul)    [128, 512]
        w[k]    = (Z[k,0:W]+margin) . oneJ  (DVE rowsum)  = pw[a_k,p_k]
        relu(-Z + w) accum -> Sacc    (ACT; includes same-label n)
        count Z < w accum  -> Cacc    (Pool; includes same-label n)
        corrections over the 128-wide window only (same-label n live
        there): relu(w-Z)*eq accum -> corrS, (Z<w)*eq accum -> corrC
Host reduces: loss = (S - corrS) / (C - corrC + EPS).

All DMA rides the single sync HWDGE queue (one queue-drain at exit);
packT is pre-packed partition-major so each partition is one contiguous
descriptor. Raw [128, T]-ish accumulators are DMA'd out; host reduces.
"""

import numpy as np
from contextlib import ExitStack

B = 512
D = 1024
NCORES = 8
A = B // NCORES   # 64 anchors per core
KT = D // 128     # 8 contraction tiles
NT = B // 128     # 4 n-tiles per anchor (dense variant)
W = 128           # per-core sentence window width
MARGIN = 0.2
EPS = 1e-16
BIG = 1e30
BIGW = 30000.0
MAXC_WIN = 33     # pair variant valid iff max class size <= this
FP8 = True        # embeddings in fp8e5m2: halves the packT DMA

_CACHE = {}


def _build_pairs(T):
    """Pair-packed kernel, raw bass (no TileContext): explicit semaphores,
    no framework drain/teardown. T tiles of 128 (anchor, positive) pairs."""
    import concourse.mybir as mybir
    from concourse import bacc

    f32 = mybir.dt.float32
    f16 = mybir.dt.float16
    Alu = mybir.AluOpType
    Act = mybir.ActivationFunctionType

    nc = bacc.Bacc("TRN2", target_bir_lowering=False, debug=False,
                   num_devices=NCORES)

    f8 = mybir.dt.float8e5
    packT_d = nc.dram_tensor("packT", [128, KT, A + B], f8 if FP8 else f16,
                             kind="ExternalInput")
    # aux regions along dim1: [0,T) oneJ | [T,2T) eqwin | [2T,3T) selA
    aux_d = nc.dram_tensor("aux", [128, 3 * T, W], f16, kind="ExternalInput")
    # out cols: [0,T) Sacc | [T,2T) count | 2T corrS | 2T+1 corrC
    out_d = nc.dram_tensor("out", [128, 2 * T + 2], f32,
                           kind="ExternalOutput")

    from contextlib import ExitStack
    with ExitStack() as ctx:
        def sb(name, shape, dt):
            return ctx.enter_context(nc.sbuf_tensor(name, shape, dt))

        def ps(name, shape, dt):
            return ctx.enter_context(nc.psum_tensor(name, shape, dt))

        def sem(name):
            return ctx.enter_context(nc.semaphore(name))

        packT = sb("packT_s", [128, KT, A + B], f8 if FP8 else f16)
        aux = sb("aux_s", [128, 3 * T, W], f16)
        zext = sb("zext", [A + 1, B], f16)
        wcol = sb("wcol", [128, T], f32)
        SC = sb("SC", [128, 2 * T + 2], f32)
        r_all = sb("r_all", [128, T, B], f16)
        junk = [sb(f"junk{t}", [128, W], f16) for t in range(T)]
        mj = [sb(f"mj{t}", [128, B], f16) for t in range(T)]
        csj = sb("csj", [128, T, W], f16)
        ccj = sb("ccj", [128, T, W], f16)
        dum = sb("dum", [1, 1], f16)

        pw_ps = ps("pw_ps", [A, B], f32)
        z_ps = [ps(f"z_ps{t}", [128, B], f32) for t in range(T)]

        sq = sem("sq")      # sync HWDGE queue: packT_lo, out
        sa = sem("sa")      # scalar HWDGE queue: packT_hi
        sw = sem("sw")      # gpsimd SWDGE: aux
        spad = sem("spad")  # zext pad row
        spw = sem("spw")    # pw matmuls done
        szx = sem("szx")    # zext rows written
        sz = sem("sz")      # z matmuls (+1 each)
        swc = sem("swc")    # wcol cols (+1 each)
        sr = sem("sr")      # relus done (+1 each)
        sv = sem("sv")      # vector post-ops done

        with nc.Block() as block:

            @block.sync
            def _(sync):
                sync.dma_start(out=packT[:, 0:4, :],
                               in_=packT_d.ap()[:, 0:4, :]).then_inc(sq, 16)
                sync.wait_ge(sr, 3)
                sync.wait_ge(sv, 1)
                sync.dma_start(out=out_d.ap(), in_=SC[:, :]).then_inc(sq, 16)
                sync.wait_ge(sq, 32)

            @block.gpsimd
            def _(gpsimd):
                gpsimd.dma_start(out=aux[:, :, :], in_=aux_d.ap()).then_inc(sw, 16)
                gpsimd.memset(zext[A:A + 1, :], BIGW).then_inc(spad, 1)

            @block.tensor
            def _(tensor):
                tensor.wait_ge(sq, 16)
                for u in range(2):
                    tensor.matmul(
                        pw_ps[:, :], lhsT=packT[:, 2 * u:2 * u + 2, 0:A],
                        rhs=packT[:, 2 * u:2 * u + 2, A:A + B],
                        start=(u == 0), stop=False,
                        perf_mode=mybir.MatmulPerfMode.DoubleRow)
                tensor.wait_ge(sa, 16)
                for u in range(2, 4):
                    mm = tensor.matmul(
                        pw_ps[:, :], lhsT=packT[:, 2 * u:2 * u + 2, 0:A],
                        rhs=packT[:, 2 * u:2 * u + 2, A:A + B],
                        start=False, stop=(u == 3),
                        perf_mode=mybir.MatmulPerfMode.DoubleRow)
                    if u == 3:
                        mm.then_inc(spw, 1)
                tensor.wait_ge(szx, 1)
                tensor.wait_ge(spad, 1)
                tensor.wait_ge(sw, 16)
                for t in range(T):
                    tensor.matmul(z_ps[t][:, :], lhsT=aux[0:A + 1, 2 * T + t, :],
                                  rhs=zext[:, :]).then_inc(sz, 1)

            @block.scalar
            def _(scalar):
                scalar.dma_start(out=packT[:, 4:8, :],
                                 in_=packT_d.ap()[:, 4:8, :]).then_inc(sa, 16)
                # dummy relu pulls the ACT table load off the critical path
                scalar.wait_ge(spad, 1)
                scalar.activation(out=dum[:, :], in_=zext[A:A + 1, 0:1],
                                  func=Act.Relu, bias=0.0, scale=1.0)
                scalar.wait_ge(spw, 1)
                scalar.activation(out=zext[0:A, :], in_=pw_ps[:, :], func=Act.Copy,
                                  bias=-MARGIN, scale=1.0).then_inc(szx, 1)
                for t in range(T):
                    scalar.wait_ge(sz, t + 1)
                    scalar.wait_ge(swc, t + 1)
                    scalar.activation(
                        out=r_all[:, t, :], in_=z_ps[t][:, :], func=Act.Relu,
                        bias=wcol[:, t:t + 1], scale=-1.0,
                        accum_out=SC[:, t:t + 1]).then_inc(sr, 1)

            @block.vector
            def _(vector):
                vector.wait_ge(sw, 16)
                for t in range(T):
                    vector.wait_ge(sz, t + 1)
                    vector.scalar_tensor_tensor(
                        junk[t][:, :], z_ps[t][:, 0:W], MARGIN, aux[:, t, :],
                        Alu.add, Alu.mult,
                        accum_out=wcol[:, t:t + 1]).then_inc(swc, 1)
                for t in range(T):
                    vector.wait_ge(sr, t + 1)
                    vector.tensor_scalar(
                        mj[t][:, :], r_all[:, t, :], 0.0, None, Alu.is_gt, Alu.add,
                        accum_out=SC[:, T + t:T + t + 1])
                vector.scalar_tensor_tensor(
                    csj[:, :, :], r_all[:, :, 0:W], 1.0, aux[:, T:2 * T, :],
                    Alu.mult, Alu.mult,
                    accum_out=SC[:, 2 * T:2 * T + 1])
                vector.scalar_tensor_tensor(
                    ccj[:, :, :], r_all[:, :, 0:W], 0.0, aux[:, T:2 * T, :],
                    Alu.is_gt, Alu.mult,
                    accum_out=SC[:, 2 * T + 1:2 * T + 2]).then_inc(sv, 1)

        nc.compile()
    return nc


def _build_dense():
    """Dense fallback (no class-size assumption)."""
    import concourse.mybir as mybir
    import concourse.tile as tile
    from concourse import bacc
    from concourse.masks import make_identity

    f32 = mybir.dt.float32
    bf16 = mybir.dt.bfloat16
    Alu = mybir.AluOpType
    Act = mybir.ActivationFunctionType
    Ax = mybir.AxisListType

    nc = bacc.Bacc("TRN2", target_bir_lowering=False, debug=False,
                   num_devices=NCORES)

    imgT_d = nc.dram_tensor("imgT", [D, A], f32, kind="ExternalInput")
    sentT_d = nc.dram_tensor("sentT", [D, B], f32, kind="ExternalInput")
    labf_d = nc.dram_tensor("labf", [B], bf16, kind="ExternalInput")
    labc_d = nc.dram_tensor("labc", [A], f32, kind="ExternalInput")
    out_d = nc.dram_tensor("out", [2], f32, kind="ExternalOutput")

    with tile.TileContext(nc) as tc:
        with ExitStack() as ctx:
            singles = ctx.enter_context(tc.tile_pool(name="singles", bufs=1))
            rpool = ctx.enter_context(tc.tile_pool(name="rpool", bufs=6))
            mpool = ctx.enter_context(tc.tile_pool(name="mpool", bufs=6))
            spsum = ctx.enter_context(
                tc.tile_pool(name="spsum", bufs=1, space="PSUM"))
            wpsum = ctx.enter_context(
                tc.tile_pool(name="wpsum", bufs=3, space="PSUM"))
            gpsum = ctx.enter_context(
                tc.tile_pool(name="gpsum", bufs=2, space="PSUM"))

            ones_r = singles.tile([1, 128], f32)
            nc.vector.memset(ones_r, 1.0)
            ones_c = singles.tile([128, 1], f32)
            nc.vector.memset(ones_c, 1.0)
            ident = singles.tile([64, 64], f32)
            make_identity(nc, ident)

            imgT = singles.tile([128, KT, A], f32)
            nc.sync.dma_start(
                out=imgT, in_=imgT_d.ap().rearrange("(t p) m -> p t m", p=128))
            sentT = singles.tile([128, KT, B], f32)
            nc.sync.dma_start(
                out=sentT, in_=sentT_d.ap().rearrange("(t p) m -> p t m", p=128))
            lab_row = singles.tile([1, B], f32)
            nc.sync.dma_start(
                out=lab_row, in_=labf_d.ap().rearrange("(o b) -> o b", o=1))
            labc_col = singles.tile([A, 1], f32)
            nc.sync.dma_start(
                out=labc_col, in_=labc_d.ap().rearrange("(a o) -> a o", o=1))

            pw_ps = spsum.tile([A, B], f32)
            for kt in range(KT):
                nc.tensor.matmul(pw_ps, lhsT=imgT[:, kt, :], rhs=sentT[:, kt, :],
                                 start=(kt == 0), stop=(kt == KT - 1))

            labB_ps = spsum.tile([A, B], f32)
            nc.tensor.matmul(labB_ps, lhsT=ones_r[:, :A], rhs=lab_row)
            eqP = singles.tile([A, B], f32)
            nc.vector.tensor_scalar(eqP, labB_ps, labc_col, None, Alu.is_equal)
            penP = singles.tile([A, B], f32)
            nc.vector.tensor_scalar(penP, eqP, 1.0, BIG, Alu.subtract, Alu.mult)
            penN = singles.tile([A, B], f32)
            nc.vector.tensor_scalar(penN, eqP, -BIG, None, Alu.mult)

            w = singles.tile([A, B], f32)
            nc.vector.tensor_scalar(w, pw_ps, MARGIN, None, Alu.add)
            nc.vector.tensor_mul(w, w, eqP)
            nc.vector.tensor_add(w, w, penP)
            negneq = singles.tile([A, B], f32)
            nc.vector.tensor_scalar(negneq, eqP, 1.0, -1.0, Alu.subtract,
                                    Alu.mult)
            z = singles.tile([A, B], f32)
            nc.vector.tensor_scalar(z, pw_ps, -1.0, None, Alu.mult)
            nc.vector.tensor_mul(z, z, negneq)
            nc.vector.tensor_add(z, z, penN)

            zTs = singles.tile([128, NT, A], f32)
            for j in range(NT):
                zt_ps = spsum.tile([128, A], f32)
                nc.tensor.transpose(zt_ps, z[:, j * 128:(j + 1) * 128], ident)
                nc.scalar.copy(zTs[:, j, :], zt_ps)

            Sacc = singles.tile([128, A * NT], f32)
            Cacc = singles.tile([128, A * NT], f32)

            for a in range(A):
                wb_ps = wpsum.tile([128, B], f32)
                nc.tensor.matmul(
                    wb_ps, lhsT=ident[:, a:a + 1].broadcast_to([A, 128]), rhs=w)
                for j in range(NT):
                    col = a * NT + j
                    r = rpool.tile([128, B], bf16)
                    nc.scalar.activation(
                        out=r, in_=wb_ps, func=Act.Relu,
                        bias=zTs[:, j, a:a + 1], scale=1.0,
                        accum_out=Sacc[:, col:col + 1])
                    m = mpool.tile([128, B], bf16)
                    nc.vector.tensor_scalar(
                        m, r, EPS, None, Alu.is_gt, Alu.add,
                        accum_out=Cacc[:, col:col + 1])

            SC = singles.tile([128, 2], f32)
            nc.vector.tensor_reduce(SC[:, 0:1], Sacc, Ax.X, Alu.add)
            nc.vector.tensor_reduce(SC[:, 1:2], Cacc, Ax.X, Alu.add)
            fin_ps = spsum.tile([2, 1], f32)
            nc.tensor.matmul(fin_ps, lhsT=SC, rhs=ones_c)
            fin_sb = singles.tile([2, 1], f32)
            nc.scalar.copy(fin_sb, fin_ps)
            nc.sync.dma_start(
                out=out_d.ap().rearrange("(p o) -> p o", o=1), in_=fin_sb)

    nc.compile()
    return nc


def _get_nc(variant, T=0):
    key = f"nc_{variant}_{T}"
    if key not in _CACHE:
        _CACHE[key] = (_build_pairs(T) if variant == "pairs"
                       else _build_dense())
    return _CACHE[key]


def _prep(labels, image_embeddings, sentence_embeddings):
    """Class-sort the batch; build per-core input maps."""
    labels = np.ascontiguousarray(labels).astype(np.int64)
    img = np.ascontiguousarray(image_embeddings, dtype=np.float32)
    sent = np.ascontiguousarray(sentence_embeddings, dtype=np.float32)
    counts = np.bincount(labels, minlength=1)
    maxc = counts.max()

    perm = np.argsort(labels, kind="stable")
    labs = labels[perm]

    if maxc > MAXC_WIN:
        imgT = np.ascontiguousarray(img[perm].T)    # [D, B]
        sentT = np.ascontiguousarray(sent[perm].T)  # [D, B]
        labsf = labs.astype(np.float32)
        maps = []
        for i in range(NCORES):
            c0 = i * A
            maps.append({
                "imgT": np.ascontiguousarray(imgT[:, c0:c0 + A]),
                "sentT": sentT,
                "labf": labsf,
                "labc": np.ascontiguousarray(labsf[c0:c0 + A]),
            })
        return "dense", 0, maps

    if FP8:
        import ml_dtypes
        edt = ml_dtypes.float8_e5m2
    else:
        edt = np.float16
    imgT = np.ascontiguousarray(img[perm].T).astype(edt)
    sentT = np.ascontiguousarray(sent[perm].T).astype(edt)

    # class run start/size per sorted position
    starts = np.concatenate([[0], np.cumsum(counts)])
    s_a = starts[labs]            # run start of each anchor
    n_a = counts[labs]            # run length of each anchor
    maxK = max(int(n_a[c0:c0 + A].sum()) for c0 in range(0, B, A))
    T = (maxK + 127) // 128
    K = T * 128

    maps = []
    for i in range(NCORES):
        c0 = i * A
        rot = (np.arange(B) + c0 - 32) % B
        # partition-major packT: [p, t, m] = flat[(t*128+p), m]
        flat = np.concatenate([imgT[:, c0:c0 + A], sentT[:, rot]], axis=1)
        packT = np.ascontiguousarray(
            flat.reshape(KT, 128, A + B).transpose(1, 0, 2))
        # aux layout: [128, K] oneJ | [128, K] eqwin | [65, K] selA
        aux = np.zeros((128, 3 * K), np.float16)
        eq = labs[rot[:W]][None, :] == labs[c0:c0 + A][:, None]  # [A, W]
        k = 0
        for a in range(A):
            ga = c0 + a
            for p in range(int(s_a[ga]), int(s_a[ga] + n_a[ga])):
                j = p - (c0 - 32)
                aux[k % 128, (k // 128) * W + j] = 1.0        # oneJ
                aux[k % 128, K + (k // 128) * W:K + (k // 128) * W + W] = \
                    eq[a].astype(np.float16)                  # eqwin row
                aux[a, 2 * K + k] = 1.0                       # selA
                k += 1
        aux[A, 2 * K + k:3 * K] = 1.0   # pads select zext row 64 (+BIGW)
        maps.append({"packT": packT, "aux": aux.reshape(128, 3 * T, W)})
    return "pairs", T, maps


def run_all(labels, image_embeddings, sentence_embeddings, trace=False):
    from concourse.bass_utils import run_bass_kernel_spmd
    variant, T, maps = _prep(labels, image_embeddings, sentence_embeddings)
    nc = _get_nc(variant, T)
    res = run_bass_kernel_spmd(nc, maps, list(range(NCORES)), trace=trace)
    if variant == "pairs":
        s = c = 0.0
        for i in range(NCORES):
            x = res.results[i]["out"]
            s += float(x[:, 0:T].sum()) - float(x[:, 2 * T].sum())
            c += float(x[:, T:2 * T].sum()) - float(x[:, 2 * T + 1].sum())
    else:
        parts = np.stack([res.results[i]["out"] for i in range(NCORES)])
        s = float(parts[:, 0].sum())
        c = float(parts[:, 1].sum())
    loss = np.float32(s / (c + EPS))
    return np.asarray(loss, dtype=np.float32), res


def kernel(labels, image_embeddings, sentence_embeddings):
    out, _ = run_all(labels, image_embeddings, sentence_embeddings)
    return out


# revision 18
# speedup vs baseline: 1.2177x; 1.0200x over previous
"""Trainium2 Bass kernel for BatchAll triplet loss.

Reference computation (B=512, D=1024):
    pw = img @ sent.T                                  [B, B]
    t[a,p,n] = pw[a,p] - pw[a,n] + margin
    valid[a,p,n] = (lab[a]==lab[p]) & (lab[a]!=lab[n])
    loss = sum(relu(valid*t)) / (count(valid*t > EPS) + EPS)

Strategy: the batch is class-sorted on the host (a pure permutation of the
(image, sentence, label) triples; the loss is permutation invariant), then
anchors are sharded across 8 cores (64 each, C = core*64). After sorting,
all positives of anchor a live in a contiguous class run inside the core's
128-wide sentence window [C-32, C+96) (holds when max class size <= 33;
dense fallback otherwise). Each core enumerates its actual valid (a,p)
pairs (sum of class sizes over its anchors, ~320 for uniform labels) and
packs them onto partitions: tiles of 128 pairs, free axis = all 512 n.

Per core, with the sentence axis pre-rotated so the window is cols [0,128):
    pw[a,n]   (PE, fp8e5m2 DoubleRow, 4 matmuls)         [64, 512]
    zext[a,n] = pw[a,n] - margin  (UNMASKED, fp16; row 64 = +30000 pad)
    per pair-tile t:
        Z[k,n]  = zext[a_k, n]        (PE one-hot matmul)    [128, 512]
        w[k]    = (Z[k,0:W]+margin) . oneJ  (DVE rowsum)  = pw[a_k,p_k]
        relu(-Z + w) accum -> Sacc    (ACT; includes same-label n)
        count Z < w accum  -> Cacc    (Pool; includes same-label n)
        corrections over the 128-wide window only (same-label n live
        there): relu(w-Z)*eq accum -> corrS, (Z<w)*eq accum -> corrC
Host reduces: loss = (S - corrS) / (C - corrC + EPS).

All DMA rides the single sync HWDGE queue (one queue-drain at exit);
packT is pre-packed partition-major so each partition is one contiguous
descriptor. Raw [128, T]-ish accumulators are DMA'd out; host reduces.
"""

import numpy as np
from contextlib import ExitStack

B = 512
D = 1024
NCORES = 8
A = B // NCORES   # 64 anchors per core
KT = D // 128     # 8 contraction tiles
NT = B // 128     # 4 n-tiles per anchor (dense variant)
W = 128           # per-core sentence window width
MARGIN = 0.2
EPS = 1e-16
BIG = 1e30
BIGW = 30000.0
MAXC_WIN = 33     # pair variant valid iff max class size <= this
FP8 = True        # embeddings in fp8e5m2: halves the packT DMA

_CACHE = {}


def _build_pairs(T):
    """Pair-packed kernel, raw bass (no TileContext): explicit semaphores,
    no framework drain/teardown. T tiles of 128 (anchor, positive) pairs."""
    import concourse.mybir as mybir
    from concourse import bacc

    f32 = mybir.dt.float32
    f16 = mybir.dt.float16
    Alu = mybir.AluOpType
    Act = mybir.ActivationFunctionType

    nc = bacc.Bacc("TRN2", target_bir_lowering=False, debug=False,
                   num_devices=NCORES)

    f8 = mybir.dt.float8e5
    packT_d = nc.dram_tensor("packT", [128, KT, A + B], f8 if FP8 else f16,
                             kind="ExternalInput")
    # aux regions along dim1: [0,T) oneJ | [T,2T) eqwin | [2T,3T) selA
    aux_d = nc.dram_tensor("aux", [128, 3 * T, W], f16, kind="ExternalInput")
    # out cols: [0,T) Sacc | [T,2T) count | [2T,3T) corrS | [3T,4T) corrC
    out_d = nc.dram_tensor("out", [128, 4 * T], f32, kind="ExternalOutput")

    from contextlib import ExitStack
    with ExitStack() as ctx:
        def sb(name, shape, dt):
            return ctx.enter_context(nc.sbuf_tensor(name, shape, dt))

        def ps(name, shape, dt):
            return ctx.enter_context(nc.psum_tensor(name, shape, dt))

        def sem(name):
            return ctx.enter_context(nc.semaphore(name))

        packT = sb("packT_s", [128, KT, A + B], f8 if FP8 else f16)
        aux = sb("aux_s", [128, 3 * T, W], f16)
        zext = sb("zext", [A + 1, B], f16)
        wcol = sb("wcol", [128, T], f32)
        SC = sb("SC", [128, 4 * T], f32)
        r_all = sb("r_all", [128, T, B], f16)
        junk = [sb(f"junk{t}", [128, W], f16) for t in range(T)]
        mj = [sb(f"mj{t}", [128, B], f16) for t in range(T)]
        csj = [sb(f"csj{t}", [128, W], f16) for t in range(T)]
        ccj = [sb(f"ccj{t}", [128, W], f16) for t in range(T)]
        dum = sb("dum", [1, 1], f16)

        pw_ps = ps("pw_ps", [A, B], f32)
        z_ps = [ps(f"z_ps{t}", [128, B], f32) for t in range(T)]

        sq1 = sem("sq1")    # sync HWDGE: packT k01
        sq2 = sem("sq2")    # sync HWDGE: packT k45
        sa1 = sem("sa1")    # scalar HWDGE: packT k23
        sa2 = sem("sa2")    # scalar HWDGE: packT k67
        so = sem("so")      # sync HWDGE: out
        sw = sem("sw")      # gpsimd SWDGE: aux
        spad = sem("spad")  # zext pad row
        spw = sem("spw")    # pw matmuls done
        szx = sem("szx")    # zext rows written
        sz = sem("sz")      # z matmuls (+1 each)
        swc = sem("swc")    # wcol cols (+1 each)
        sr = sem("sr")      # relus done (+1 each)
        sv = sem("sv")      # vector post-ops done

        with nc.Block(no_gpsimd_drain=True) as block:

            @block.sync
            def _(sync):
                sync.dma_start(out=packT[:, 0:2, :],
                               in_=packT_d.ap()[:, 0:2, :]).then_inc(sq1, 16)
                sync.dma_start(out=packT[:, 4:6, :],
                               in_=packT_d.ap()[:, 4:6, :]).then_inc(sq2, 16)
                sync.wait_ge(sr, 3)
                sync.wait_ge(sv, 1)
                sync.dma_start(out=out_d.ap(), in_=SC[:, :],
                               single_packet=True).then_inc(so, 16)
                sync.wait_ge(so, 16)

            @block.gpsimd
            def _(gpsimd):
                gpsimd.dma_start(out=aux[:, :, :], in_=aux_d.ap()).then_inc(sw, 16)
                gpsimd.memset(zext[A:A + 1, :], BIGW).then_inc(spad, 1)

            @block.tensor
            def _(tensor):
                # chunk order k01(sync) k23(scalar) k45(sync) k67(scalar):
                # each chunk's completion crawl overlaps the previous matmul
                waits = [(sq1, 16), (sa1, 16), (sq2, 16), (sa2, 16)]
                chunks = [0, 1, 2, 3]
                for i, u in enumerate(chunks):
                    tensor.wait_ge(*waits[i])
                    mm = tensor.matmul(
                        pw_ps[:, :], lhsT=packT[:, 2 * u:2 * u + 2, 0:A],
                        rhs=packT[:, 2 * u:2 * u + 2, A:A + B],
                        start=(i == 0), stop=(i == 3),
                        perf_mode=mybir.MatmulPerfMode.DoubleRow)
                    if i == 3:
                        mm.then_inc(spw, 1)
                tensor.wait_ge(szx, 1)
                tensor.wait_ge(spad, 1)
                tensor.wait_ge(sw, 16)
                for t in range(T):
                    tensor.matmul(z_ps[t][:, :], lhsT=aux[0:A + 1, 2 * T + t, :],
                                  rhs=zext[:, :]).then_inc(sz, 1)

            @block.scalar
            def _(scalar):
                scalar.dma_start(out=packT[:, 2:4, :],
                                 in_=packT_d.ap()[:, 2:4, :]).then_inc(sa1, 16)
                scalar.dma_start(out=packT[:, 6:8, :],
                                 in_=packT_d.ap()[:, 6:8, :]).then_inc(sa2, 16)
                # dummy relu pulls the ACT table load off the critical path
                scalar.wait_ge(spad, 1)
                scalar.activation(out=dum[:, :], in_=zext[A:A + 1, 0:1],
                                  func=Act.Relu, bias=0.0, scale=1.0)
                scalar.wait_ge(spw, 1)
                scalar.activation(out=zext[0:A, :], in_=pw_ps[:, :], func=Act.Copy,
                                  bias=-MARGIN, scale=1.0).then_inc(szx, 1)
                for t in range(T):
                    scalar.wait_ge(sz, t + 1)
                    scalar.wait_ge(swc, t + 1)
                    scalar.activation(
                        out=r_all[:, t, :], in_=z_ps[t][:, :], func=Act.Relu,
                        bias=wcol[:, t:t + 1], scale=-1.0,
                        accum_out=SC[:, t:t + 1]).then_inc(sr, 1)

            @block.vector
            def _(vector):
                vector.wait_ge(sw, 16)
                for t in range(T):
                    vector.wait_ge(sz, t + 1)
                    vector.scalar_tensor_tensor(
                        junk[t][:, :], z_ps[t][:, 0:W], MARGIN, aux[:, t, :],
                        Alu.add, Alu.mult,
                        accum_out=wcol[:, t:t + 1]).then_inc(swc, 1)
                for t in range(T):
                    vector.wait_ge(sr, t + 1)
                    vector.tensor_scalar(
                        mj[t][:, :], r_all[:, t, :], 0.0, None, Alu.is_gt,
                        Alu.add, accum_out=SC[:, T + t:T + t + 1])
                    vector.scalar_tensor_tensor(
                        csj[t][:, :], r_all[:, t, 0:W], 1.0,
                        aux[:, T + t, :], Alu.mult, Alu.mult,
                        accum_out=SC[:, 2 * T + t:2 * T + t + 1])
                    cc = vector.scalar_tensor_tensor(
                        ccj[t][:, :], r_all[:, t, 0:W], 0.0,
                        aux[:, T + t, :], Alu.is_gt, Alu.mult,
                        accum_out=SC[:, 3 * T + t:3 * T + t + 1])
                    if t == T - 1:
                        cc.then_inc(sv, 1)

        nc.compile()
    return nc


def _build_dense():
    """Dense fallback (no class-size assumption)."""
    import concourse.mybir as mybir
    import concourse.tile as tile
    from concourse import bacc
    from concourse.masks import make_identity

    f32 = mybir.dt.float32
    bf16 = mybir.dt.bfloat16
    Alu = mybir.AluOpType
    Act = mybir.ActivationFunctionType
    Ax = mybir.AxisListType

    nc = bacc.Bacc("TRN2", target_bir_lowering=False, debug=False,
                   num_devices=NCORES)

    imgT_d = nc.dram_tensor("imgT", [D, A], f32, kind="ExternalInput")
    sentT_d = nc.dram_tensor("sentT", [D, B], f32, kind="ExternalInput")
    labf_d = nc.dram_tensor("labf", [B], bf16, kind="ExternalInput")
    labc_d = nc.dram_tensor("labc", [A], f32, kind="ExternalInput")
    out_d = nc.dram_tensor("out", [2], f32, kind="ExternalOutput")

    with tile.TileContext(nc) as tc:
        with ExitStack() as ctx:
            singles = ctx.enter_context(tc.tile_pool(name="singles", bufs=1))
            rpool = ctx.enter_context(tc.tile_pool(name="rpool", bufs=6))
            mpool = ctx.enter_context(tc.tile_pool(name="mpool", bufs=6))
            spsum = ctx.enter_context(
                tc.tile_pool(name="spsum", bufs=1, space="PSUM"))
            wpsum = ctx.enter_context(
                tc.tile_pool(name="wpsum", bufs=3, space="PSUM"))
            gpsum = ctx.enter_context(
                tc.tile_pool(name="gpsum", bufs=2, space="PSUM"))

            ones_r = singles.tile([1, 128], f32)
            nc.vector.memset(ones_r, 1.0)
            ones_c = singles.tile([128, 1], f32)
            nc.vector.memset(ones_c, 1.0)
            ident = singles.tile([64, 64], f32)
            make_identity(nc, ident)

            imgT = singles.tile([128, KT, A], f32)
            nc.sync.dma_start(
                out=imgT, in_=imgT_d.ap().rearrange("(t p) m -> p t m", p=128))
            sentT = singles.tile([128, KT, B], f32)
            nc.sync.dma_start(
                out=sentT, in_=sentT_d.ap().rearrange("(t p) m -> p t m", p=128))
            lab_row = singles.tile([1, B], f32)
            nc.sync.dma_start(
                out=lab_row, in_=labf_d.ap().rearrange("(o b) -> o b", o=1))
            labc_col = singles.tile([A, 1], f32)
            nc.sync.dma_start(
                out=labc_col, in_=labc_d.ap().rearrange("(a o) -> a o", o=1))

            pw_ps = spsum.tile([A, B], f32)
            for kt in range(KT):
                nc.tensor.matmul(pw_ps, lhsT=imgT[:, kt, :], rhs=sentT[:, kt, :],
                                 start=(kt == 0), stop=(kt == KT - 1))

            labB_ps = spsum.tile([A, B], f32)
            nc.tensor.matmul(labB_ps, lhsT=ones_r[:, :A], rhs=lab_row)
            eqP = singles.tile([A, B], f32)
            nc.vector.tensor_scalar(eqP, labB_ps, labc_col, None, Alu.is_equal)
            penP = singles.tile([A, B], f32)
            nc.vector.tensor_scalar(penP, eqP, 1.0, BIG, Alu.subtract, Alu.mult)
            penN = singles.tile([A, B], f32)
            nc.vector.tensor_scalar(penN, eqP, -BIG, None, Alu.mult)

            w = singles.tile([A, B], f32)
            nc.vector.tensor_scalar(w, pw_ps, MARGIN, None, Alu.add)
            nc.vector.tensor_mul(w, w, eqP)
            nc.vector.tensor_add(w, w, penP)
            negneq = singles.tile([A, B], f32)
            nc.vector.tensor_scalar(negneq, eqP, 1.0, -1.0, Alu.subtract,
                                    Alu.mult)
            z = singles.tile([A, B], f32)
            nc.vector.tensor_scalar(z, pw_ps, -1.0, None, Alu.mult)
            nc.vector.tensor_mul(z, z, negneq)
            nc.vector.tensor_add(z, z, penN)

            zTs = singles.tile([128, NT, A], f32)
            for j in range(NT):
                zt_ps = spsum.tile([128, A], f32)
                nc.tensor.transpose(zt_ps, z[:, j * 128:(j + 1) * 128], ident)
                nc.scalar.copy(zTs[:, j, :], zt_ps)

            Sacc = singles.tile([128, A * NT], f32)
            Cacc = singles.tile([128, A * NT], f32)

            for a in range(A):
                wb_ps = wpsum.tile([128, B], f32)
                nc.tensor.matmul(
                    wb_ps, lhsT=ident[:, a:a + 1].broadcast_to([A, 128]), rhs=w)
                for j in range(NT):
                    col = a * NT + j
                    r = rpool.tile([128, B], bf16)
                    nc.scalar.activation(
                        out=r, in_=wb_ps, func=Act.Relu,
                        bias=zTs[:, j, a:a + 1], scale=1.0,
                        accum_out=Sacc[:, col:col + 1])
                    m = mpool.tile([128, B], bf16)
                    nc.vector.tensor_scalar(
                        m, r, EPS, None, Alu.is_gt, Alu.add,
                        accum_out=Cacc[:, col:col + 1])

            SC = singles.tile([128, 2], f32)
            nc.vector.tensor_reduce(SC[:, 0:1], Sacc, Ax.X, Alu.add)
            nc.vector.tensor_reduce(SC[:, 1:2], Cacc, Ax.X, Alu.add)
            fin_ps = spsum.tile([2, 1], f32)
            nc.tensor.matmul(fin_ps, lhsT=SC, rhs=ones_c)
            fin_sb = singles.tile([2, 1], f32)
            nc.scalar.copy(fin_sb, fin_ps)
            nc.sync.dma_start(
                out=out_d.ap().rearrange("(p o) -> p o", o=1), in_=fin_sb)

    nc.compile()
    return nc


def _get_nc(variant, T=0):
    key = f"nc_{variant}_{T}"
    if key not in _CACHE:
        _CACHE[key] = (_build_pairs(T) if variant == "pairs"
                       else _build_dense())
    return _CACHE[key]


def _prep(labels, image_embeddings, sentence_embeddings):
    """Class-sort the batch; build per-core input maps."""
    labels = np.ascontiguousarray(labels).astype(np.int64)
    img = np.ascontiguousarray(image_embeddings, dtype=np.float32)
    sent = np.ascontiguousarray(sentence_embeddings, dtype=np.float32)
    counts = np.bincount(labels, minlength=1)
    maxc = counts.max()

    perm = np.argsort(labels, kind="stable")
    labs = labels[perm]

    if maxc > MAXC_WIN:
        imgT = np.ascontiguousarray(img[perm].T)    # [D, B]
        sentT = np.ascontiguousarray(sent[perm].T)  # [D, B]
        labsf = labs.astype(np.float32)
        maps = []
        for i in range(NCORES):
            c0 = i * A
            maps.append({
                "imgT": np.ascontiguousarray(imgT[:, c0:c0 + A]),
                "sentT": sentT,
                "labf": labsf,
                "labc": np.ascontiguousarray(labsf[c0:c0 + A]),
            })
        return "dense", 0, maps

    if FP8:
        import ml_dtypes
        edt = ml_dtypes.float8_e5m2
    else:
        edt = np.float16
    imgT = np.ascontiguousarray(img[perm].T).astype(edt)
    sentT = np.ascontiguousarray(sent[perm].T).astype(edt)

    # class run start/size per sorted position
    starts = np.concatenate([[0], np.cumsum(counts)])
    s_a = starts[labs]            # run start of each anchor
    n_a = counts[labs]            # run length of each anchor
    maxK = max(int(n_a[c0:c0 + A].sum()) for c0 in range(0, B, A))
    T = (maxK + 127) // 128
    K = T * 128

    maps = []
    for i in range(NCORES):
        c0 = i * A
        rot = (np.arange(B) + c0 - 32) % B
        # partition-major packT: [p, t, m] = flat[(t*128+p), m]
        flat = np.concatenate([imgT[:, c0:c0 + A], sentT[:, rot]], axis=1)
        packT = np.ascontiguousarray(
            flat.reshape(KT, 128, A + B).transpose(1, 0, 2))
        # aux layout: [128, K] oneJ | [128, K] eqwin | [65, K] selA
        aux = np.zeros((128, 3 * K), np.float16)
        eq = labs[rot[:W]][None, :] == labs[c0:c0 + A][:, None]  # [A, W]
        k = 0
        for a in range(A):
            ga = c0 + a
            for p in range(int(s_a[ga]), int(s_a[ga] + n_a[ga])):
                j = p - (c0 - 32)
                aux[k % 128, (k // 128) * W + j] = 1.0        # oneJ
                aux[k % 128, K + (k // 128) * W:K + (k // 128) * W + W] = \
                    eq[a].astype(np.float16)                  # eqwin row
                aux[a, 2 * K + k] = 1.0                       # selA
                k += 1
        aux[A, 2 * K + k:3 * K] = 1.0   # pads select zext row 64 (+BIGW)
        maps.append({"packT": packT, "aux": aux.reshape(128, 3 * T, W)})
    return "pairs", T, maps


def run_all(labels, image_embeddings, sentence_embeddings, trace=False):
    from concourse.bass_utils import run_bass_kernel_spmd
    variant, T, maps = _prep(labels, image_embeddings, sentence_embeddings)
    nc = _get_nc(variant, T)
    res = run_bass_kernel_spmd(nc, maps, list(range(NCORES)), trace=trace)
    if variant == "pairs":
        s = c = 0.0
        for i in range(NCORES):
            x = res.results[i]["out"]
            s += float(x[:, 0:T].sum()) - float(x[:, 2 * T:3 * T].sum())
            c += float(x[:, T:2 * T].sum()) - float(x[:, 3 * T:].sum())
    else:
        parts = np.stack([res.results[i]["out"] for i in range(NCORES)])
        s = float(parts[:, 0].sum())
        c = float(parts[:, 1].sum())
    loss = np.float32(s / (c + EPS))
    return np.asarray(loss, dtype=np.float32), res


def kernel(labels, image_embeddings, sentence_embeddings):
    out, _ = run_all(labels, image_embeddings, sentence_embeddings)
    return out
